# revision 41
# baseline (speedup 1.0000x reference)
"""Cost-volume kernel for Trainium2 (8 NeuronCores, batch-parallel).

Problem: cost[b, o=(dy,dx), h, w] = PReLU(mean_c(c1[b,c,h,w] *
         pad(warped)[b,c,h+dy,w+dx]), alpha), 81 offsets (9x9), zero pad 4.

Strategy per core (one batch element per NeuronCore):
  - Image tiled 16x8 pixels, M=128 pixel tile, m = b8*16 + a.
  - TensorE gram tile vs the 24x16 warped halo: PSUM[m, n] =
    sum_c c1[c, p_m] * wpad[c, halo_n] (K=96+96, N=384, bf16->fp32).
  - The 81 cost entries of pixel (a, b8) live at n = (a+dy)*16 + (b8+dx),
    a sheared per-partition window no SBUF AP can express, so the device
    writes the partition-uniform 144-superset per row-group a and the
    host finishes with a numpy diagonal gather + PReLU + 1/192 scale.

Scheduling (v21), from trace-measured DMA behavior (16 SDMA engines;
~16.5 B/ns/engine on reads regardless of DMA size; scattered-run write
rate scales with run length; SWDGE Q7 issue ~0.65us/DMA dominated by
per-DMA fixed cost; HWDGE write packets pin to engines 0-7; an HWDGE
gout DMA blocks its issuing sequencer in program order, so the scalar
ring -- which doubles as the ACT copy engine -- must stay gout-free
until the final band, and sync tolerates only a bounded share while
wpad loads remain):
  - QUAD-interleaved staging: each PSUM tile [128,2048] (4 banks, 2
    bufs) holds 4 tiles' grams; gout runs are 4*WIN=576 elements
    (1152B) and descriptor counts halve vs pair interleaving.
  - Each quad's PSUM->SBUF cast copy is split into two concurrent
    half-copies (DVE n<192, ACT n>=192), so the copy latency stays
    ~0.9us and the 2-buffer PSUM ring never stalls the PE.
  - Reads stay fine-grained for just-in-time pacing (12-row wpad chunks
    on sync, per-band c1 on scalar, 3 bands of prefetch, k0 halves
    first feeding a k0-first matmul lead-in on band 0).
  - staged SBUF is one persistent tile of 4 per-band slots with
    range-granular WAR deps; gout waves: bands 0-3 per-band on Q7 (+2
    sync) while Q7 is otherwise idle; group 2 half-sync; band 7 in two
    half-waves, the last on sync+scalar, balanced across sequencers.
"""

import numpy as np

B, C, H, W = 8, 192, 128, 160
R = 4
TH, TW = 16, 8                    # pixel tile
HH, HWW = TH + 2 * R, TW + 2 * R  # halo 24 x 16
NCOL = HH * HWW                   # 384 matmul free dim
BANDS = H // TH                   # 8 row bands
TPB = W // TW                     # 20 tiles per band
WIN = 2 * R * HWW + TW + 2 * R    # 144 per-a superset window
PH, PW = H + 2 * R, W + 2 * R     # padded 136 x 168
K0, K1 = 96, 96                   # contraction chunks
GB = 2                            # bands per slot-pair group
NGRP = BANDS // GB                # 4 groups
NQB = TPB // 4                    # 5 tile-quads per band
GQUAD = GB * NQB                  # 10 quads per group
GROW = TW * GQUAD * 4 * WIN       # 46080 cols per gout row

_CACHE = {}


def _build():
    if "nc" in _CACHE:
        return _CACHE["nc"]
    import sys
    if "/opt/trn_rl_repo" not in sys.path:
        sys.path.insert(0, "/opt/trn_rl_repo")
    import concourse.mybir as mybir
    import concourse.tile as tile
    from concourse import bacc
    from concourse.bass import AP

    nc = bacc.Bacc(None, target_bir_lowering=False)
    bf16 = mybir.dt.bfloat16
    f32 = mybir.dt.float32

    # c1 pre-tiled on host: [C, band, t, m], m = b8*16 + a
    c1_d = nc.dram_tensor("c1b", [C, H * W], bf16, kind="ExternalInput")
    wp_d = nc.dram_tensor("wpad", [C, PH * PW], bf16, kind="ExternalInput")
    go_d = nc.dram_tensor("gout", [NGRP * TH, GROW], bf16,
                          kind="ExternalOutput")

    with tile.TileContext(nc) as tc:
        with (
            tc.tile_pool(name="wp", bufs=1) as wp_pool,
            tc.tile_pool(name="c1", bufs=4) as c1_pool,
            tc.tile_pool(name="st", bufs=1) as st_pool,
            tc.tile_pool(name="ps", bufs=2, space="PSUM") as ps_pool,
        ):
            # one persistent padded-warped tile per channel chunk; rows
            # [4,132) are real data, rows [0,4) and [132,136) are pad.
            wp_sb = {}
            for k, kn in enumerate((K0, K1)):
                t = wp_pool.tile([kn, PH * PW], bf16, tag=f"wp{k}")
                wp_sb[k] = t
                nc.gpsimd.memset(t[:, 0:R * PW], 0.0)
                nc.gpsimd.memset(t[:, (PH - R) * PW:PH * PW], 0.0)

            # 12-row chunks: fine-grained deps pace band compute
            WCHUNKS = tuple((r, min(r + 12, PH - R))
                            for r in range(R, PH - R, 12))

            def load_wp_chunk(ci):
                ra, rb = WCHUNKS[ci]
                for k, (ks, kn) in enumerate(((0, K0), (K0, K1))):
                    nc.sync.dma_start(
                        wp_sb[k][:, ra * PW:rb * PW],
                        wp_d[ks:ks + kn, ra * PW:rb * PW])

            def load_c1(band, eng):
                tiles = []
                for k, (ks, kn) in enumerate(((0, K0), (K0, K1))):
                    t = c1_pool.tile([kn, TPB * 128], bf16, tag=f"c1_{k}")
                    eng.dma_start(
                        t[:], c1_d[ks:ks + kn,
                                   band * TPB * 128:(band + 1) * TPB * 128])
                    tiles.append(t)
                return tiles

            # band n's halo rows [16n, 16n+24) live in chunks with
            # ra < 16n+24; pace loads two bands ahead of compute
            def wmax(n):
                return max(ci for ci, (ra, rb) in enumerate(WCHUNKS)
                           if ra < 16 * n + 24)

            # k0 halves first so band 0's k0-first matmul lead-in can
            # start as early as possible
            def load_wp_half(ci, k):
                ra, rb = WCHUNKS[ci]
                ks, kn = ((0, K0), (K0, K1))[k]
                nc.sync.dma_start(wp_sb[k][:, ra * PW:rb * PW],
                                  wp_d[ks:ks + kn, ra * PW:rb * PW])

            def load_c1_half(band, k, tiles):
                ks, kn = ((0, K0), (K0, K1))[k]
                t = c1_pool.tile([kn, TPB * 128], bf16, tag=f"c1_{k}")
                nc.scalar.dma_start(
                    t[:], c1_d[ks:ks + kn,
                               band * TPB * 128:(band + 1) * TPB * 128])
                tiles.append(t)

            b0 = []
            load_wp_half(0, 0)
            load_wp_half(1, 0)
            load_c1_half(0, 0, b0)
            load_wp_half(0, 1)
            load_wp_half(1, 1)
            load_c1_half(0, 1, b0)
            c1_tiles = {0: b0}
            c1_tiles[1] = load_c1(1, nc.scalar)
            load_wp_chunk(2)
            c1_tiles[2] = load_c1(2, nc.scalar)
            wp_next = 3

            # PE warm-up burst during the initial DMA window keeps the HAM
            # clock up before the real stream starts.
            warm = c1_pool.tile([128, 512], bf16, tag="warm")
            nc.gpsimd.memset(warm[:], 0.0)
            for _ in range(10):
                ps_w = ps_pool.tile([128, 2048], f32, tag="ps")
                nc.tensor.matmul(ps_w[:, 0:512], warm[:, 0:128],
                                 warm[:, 0:512], start=True, stop=True)

            # one persistent staged tile = ring of 4 per-band slots;
            # range-granular deps mean band b's copies wait only on band
            # b-4's gout reads, not a whole pool buffer (no group stall)
            SLOT = NQB * 4 * NCOL
            staged = st_pool.tile([128, 4 * SLOT], bf16, tag="staged")
            sap0 = staged[:]
            srow = sap0.ap[0][0]
            gap = go_d[:]

            def emit_wave(grp_, gbase_, qlo, qhi, sel):
                # one gout DMA per row-group a covering group-quad range
                # [qlo, qhi); runs of 4*WIN=576 elements (1152B)
                nq = qhi - qlo
                for a in range(TH):
                    src = AP(sap0.tensor,
                             sap0.offset + gbase_ + qlo * 4 * NCOL
                             + a * srow + 4 * a * HWW,
                             [[TH * srow, TW], [4 * NCOL, nq],
                              [1, 4 * WIN]])
                    dst = AP(gap.tensor,
                             gap.offset + (grp_ * TH + a) * GROW
                             + qlo * 4 * WIN,
                             [[GQUAD * 4 * WIN, TW], [4 * WIN, nq],
                              [1, 4 * WIN]])
                    sel(a).dma_start(dst, src)

            for grp in range(NGRP):
                gbase = (grp % 2) * 2 * SLOT  # slots {0,1} or {2,3}

                for bb in range(GB):
                    band = grp * GB + bb
                    r0 = band * TH
                    c1_sb = c1_tiles.pop(band)
                    # prefetch three bands ahead; pace wp chunks likewise
                    if band + 3 < BANDS:
                        c1_tiles[band + 3] = load_c1(band + 3, nc.scalar)
                    while wp_next <= wmax(min(band + 2, BANDS - 1)):
                        load_wp_chunk(wp_next)
                        wp_next += 1

                    def mm(ps, qb, sub, k, start, stop):
                        t_i = 4 * qb + sub
                        c0 = t_i * TW
                        kn = (K0, K1)[k]
                        a1 = c1_sb[k][:]
                        lhsT = AP(a1.tensor, a1.offset + t_i * 128,
                                  [[a1.ap[0][0], kn], [1, 128]])
                        a2 = wp_sb[k][:]
                        rhs = AP(a2.tensor, a2.offset + r0 * PW + c0,
                                 [[a2.ap[0][0], kn], [PW, HH], [1, HWW]])
                        nc.tensor.matmul(
                            ps[:, sub * 512:sub * 512 + NCOL], lhsT, rhs,
                            start=start, stop=stop)

                    def docopy(ps, qb):
                        # two concurrent half-copies (DVE + ACT) move the
                        # quad's grams n-major/quad-inner (staged col
                        # j = 4n + sub) in ~0.9us, so the 2-buffer PSUM
                        # ring never gates the PE
                        pap = ps[:]
                        d0 = gbase + (bb * NQB + qb) * 4 * NCOL
                        for h, eng in ((0, nc.vector.tensor_copy),
                                       (1, nc.scalar.copy)):
                            n0 = h * (NCOL // 2)
                            src2 = AP(pap.tensor, pap.offset + n0,
                                      [[pap.ap[0][0], 128],
                                       [1, NCOL // 2], [512, 4]])
                            dst2 = staged[:, d0 + 4 * n0:
                                          d0 + 4 * n0 + 2 * NCOL]
                            eng(dst2, src2)

                    if band == 0:
                        # k0-first lead-in: the k1 chunks land later on
                        # the rings, so run the first 2 quads' k0
                        # matmuls before any k1
                        pss = []
                        for qb in range(2):
                            ps = ps_pool.tile([128, 2048], f32, tag="ps")
                            pss.append(ps)
                            for sub in range(4):
                                mm(ps, qb, sub, 0, True, False)
                        for qb in range(2):
                            ps = pss[qb]
                            for sub in range(4):
                                mm(ps, qb, sub, 1, False, True)
                            docopy(ps, qb)
                        qrange = range(2, NQB)
                    else:
                        qrange = range(NQB)

                    for qb in qrange:
                        ps = ps_pool.tile([128, 2048], f32, tag="ps")
                        for sub in range(4):
                            for k in range(2):
                                mm(ps, qb, sub, k, k == 0, k == 1)
                        docopy(ps, qb)
                        if band == BANDS - 1 and qb == 2:
                            # band-7 first half-wave mid-band: the drain
                            # tail shrinks by ~half a band
                            emit_wave(grp, gbase, NQB, NQB + 3,
                                      lambda a: (nc.sync,
                                                 nc.gpsimd)[a % 2])

                    if band < 4:
                        # Q7 is otherwise idle here: per-band waves free
                        # the staged slots ~10us earlier, removing the
                        # slot-WAR stalls that gate bands 4-6's copies
                        emit_wave(grp, gbase, bb * NQB, (bb + 1) * NQB,
                                  lambda a: nc.sync if a % 8 == 2
                                  else nc.gpsimd)
                    elif grp < NGRP - 1:
                        if bb == GB - 1:
                            # 2-band wave at group end; reads are nearly
                            # drained: the sync ring takes half,
                            # relieving the SWDGE FIFO whose serial
                            # drain sets the tail
                            emit_wave(grp, gbase, 0, GQUAD,
                                      lambda a: nc.sync if a % 2 == 0
                                      else nc.gpsimd)
                    elif bb == 0:
                        # band 6: wave right after its copies, draining
                        # through band 7's compute
                        emit_wave(grp, gbase, 0, NQB,
                                  lambda a: (nc.sync, nc.gpsimd,
                                             nc.sync, nc.gpsimd,
                                             nc.gpsimd, nc.sync)[a % 6])
                    else:
                        # band 7 second half-wave: last copies done, both
                        # HWDGE rings free; even split keeps the two
                        # sequencers' serial issue chains balanced
                        emit_wave(grp, gbase, NQB + 3, GQUAD,
                                  lambda a: (nc.sync, nc.scalar)[a % 2])

    nc.finalize()
    _CACHE["nc"] = nc
    return nc


def kernel(c1, warped, alpha):
    import sys
    if "/opt/trn_rl_repo" not in sys.path:
        sys.path.insert(0, "/opt/trn_rl_repo")
    import ml_dtypes
    from concourse.bass_utils import run_bass_kernel_spmd

    nc = _build()
    bf = ml_dtypes.bfloat16

    in_maps = []
    for b in range(B):
        wpad = np.zeros((C, PH, PW), np.float32)
        wpad[:, R:R + H, R:R + W] = warped[b]
        # tile c1: [C, band, a, t, b8] -> [C, band, t, b8, a]; m = b8*16 + a
        c1t = np.asarray(c1[b]).reshape(C, BANDS, TH, TPB, TW)
        c1t = np.ascontiguousarray(c1t.transpose(0, 1, 3, 4, 2))
        in_maps.append({
            "c1b": c1t.reshape(C, H * W).astype(bf),
            "wpad": wpad.reshape(C, PH * PW).astype(bf),
        })

    import os
    trace = bool(int(os.environ.get("COSTVOL_TRACE", "0")))
    res = run_bass_kernel_spmd(nc, in_maps, core_ids=list(range(B)),
                               trace=trace)
    if trace:
        _CACHE["last_exec_time_ns"] = res.exec_time_ns

    # host-side: de-interleave + diagonal gather + mean + PReLU
    a_val = float(np.asarray(alpha).reshape(-1)[0])
    dy, dx = np.meshgrid(np.arange(9), np.arange(9), indexing="ij")
    oidx = (dy * HWW + dx).reshape(-1)                      # [81]
    # gout row (grp*16+a) cols: [b8][quad(bb,qb)][j],
    # j = 4*(16dy+b8+dx) + sub, sub = tile%4
    jidx = (4 * (np.arange(TW)[:, None, None] + oidx[None, None, :])
            + np.arange(4)[None, :, None])                  # [b8, sub, 81]
    jflat = jidx.reshape(TW, 4 * 81)                        # [b8, 324]

    out = np.empty((B, 81, H, W), np.float32)
    for b in range(B):
        g = np.asarray(res.results[b]["gout"]).astype(np.float32)
        g = g.reshape(NGRP, TH, TW, GB, NQB, 4 * WIN)
        got = np.take_along_axis(
            g, jflat[None, None, :, None, None, :], axis=5)
        got = got.reshape(NGRP, TH, TW, GB, NQB, 4, 81)
        # axes [grp, a, b8, bb, qb, sub, o] -> [o, grp, bb, a, qb, sub, b8]
        cost = got.transpose(6, 0, 3, 1, 4, 5, 2).reshape(81, H, W) \
            * (1.0 / C)
        out[b] = np.where(cost >= 0, cost, a_val * cost)
    return out


# revision 42
# speedup vs baseline: 1.0811x; 1.0811x over previous
"""Cost-volume kernel for Trainium2 (8 NeuronCores, batch-parallel).

Problem: cost[b, o=(dy,dx), h, w] = PReLU(mean_c(c1[b,c,h,w] *
         pad(warped)[b,c,h+dy,w+dx]), alpha), 81 offsets (9x9), zero pad 4.

Strategy per core (one batch element per NeuronCore):
  - Image tiled 16x8 pixels, M=128 pixel tile, m = b8*16 + a.
  - TensorE gram tile vs the 24x16 warped halo: PSUM[m, n] =
    sum_c c1[c, p_m] * wpad[c, halo_n] (K=96+96, N=384, bf16->fp32).
  - The 81 cost entries of pixel (a, b8) live at n = (a+dy)*16 + (b8+dx),
    a sheared per-partition window no SBUF AP can express, so the device
    writes the partition-uniform 144-superset per row-group a and the
    host finishes with a numpy diagonal gather + PReLU + 1/192 scale.

Scheduling (v17), from trace-measured DMA behavior (16 SDMA engines;
~16.5 B/ns/engine on reads regardless of DMA size; scattered-run write
rate scales with run length: 288B runs ~9-10, 576B runs ~15 B/ns;
SWDGE Q7 issue ~0.65us/DMA fixed, HWDGE ~0.45us; HWDGE write packets
pin to engines 0-7, SWDGE spreads over all 16; an HWDGE gout DMA
blocks its issuing sequencer in program order, so the scalar ring --
which doubles as the ACT copy engine -- must stay gout-free until the
final band, and the sync ring tolerates only a bounded share while
wpad loads remain):
  - PSUM->SBUF cast copy reads the tile-pair's gram columns n-major/
    hf-inner, pair-interleaving staged SBUF at zero copy cost (f32 PSUM
    reads have no contiguity bonus to lose) -> gout runs 576B.
  - Reads stay fine-grained for just-in-time pacing (12-row wpad chunks
    on sync, per-band c1 on scalar, 3 bands of prefetch, k0 halves
    first feeding a k0-first matmul lead-in on band 0): coarse 1MB
    chunks bubble the matmul pipe at the same engine rate.
  - staged SBUF is one persistent tile of 4 per-band slots with
    range-granular WAR deps; gout waves: bands 0-3 per-band on Q7 (+2
    sync) while Q7 is otherwise idle, freeing slots early; groups 2-3
    at coarser grain with progressively more HWDGE share as the read
    streams drain (g2: half sync; band 7 in two half-waves, the last
    one on sync+scalar).
"""

import numpy as np

B, C, H, W = 8, 192, 128, 160
R = 4
TH, TW = 16, 8                    # pixel tile
HH, HWW = TH + 2 * R, TW + 2 * R  # halo 24 x 16
NCOL = HH * HWW                   # 384 matmul free dim
BANDS = H // TH                   # 8 row bands
TPB = W // TW                     # 20 tiles per band
WIN = 2 * R * HWW + TW + 2 * R    # 144 per-a superset window
PH, PW = H + 2 * R, W + 2 * R     # padded 136 x 168
K0, K1 = 96, 96                   # contraction chunks
GB = 2                            # bands per staged group
NGRP = BANDS // GB                # 4 staged groups
NPAIR = TPB // 2                  # tile-pairs per band
GPAIR = GB * NPAIR                # 20 pairs per group
GROW = TW * GPAIR * 2 * WIN       # 46080 cols per gout row

_CACHE = {}


def _build():
    if "nc" in _CACHE:
        return _CACHE["nc"]
    import sys
    if "/opt/trn_rl_repo" not in sys.path:
        sys.path.insert(0, "/opt/trn_rl_repo")
    import concourse.mybir as mybir
    import concourse.tile as tile
    from concourse import bacc
    from concourse.bass import AP

    nc = bacc.Bacc(None, target_bir_lowering=False)
    bf16 = mybir.dt.bfloat16
    f32 = mybir.dt.float32

    # c1 pre-tiled on host: [C, band, t, m], m = b8*16 + a
    c1_d = nc.dram_tensor("c1b", [C, H * W], bf16, kind="ExternalInput")
    wp_d = nc.dram_tensor("wpad", [C, PH * PW], bf16, kind="ExternalInput")
    go_d = nc.dram_tensor("gout", [NGRP * TH, GROW], bf16,
                          kind="ExternalOutput")

    with tile.TileContext(nc) as tc:
        with (
            tc.tile_pool(name="wp", bufs=1) as wp_pool,
            tc.tile_pool(name="c1", bufs=4) as c1_pool,
            tc.tile_pool(name="st", bufs=1) as st_pool,
            tc.tile_pool(name="ps", bufs=4, space="PSUM") as ps_pool,
        ):
            # one persistent padded-warped tile per channel chunk; rows
            # [4,132) are real data, rows [0,4) and [132,136) are pad.
            wp_sb = {}
            for k, kn in enumerate((K0, K1)):
                t = wp_pool.tile([kn, PH * PW], bf16, tag=f"wp{k}")
                wp_sb[k] = t
                nc.gpsimd.memset(t[:, 0:R * PW], 0.0)
                nc.gpsimd.memset(t[:, (PH - R) * PW:PH * PW], 0.0)

            # 12-row chunks: fine-grained deps pace band compute
            WCHUNKS = tuple((r, min(r + 12, PH - R))
                            for r in range(R, PH - R, 12))

            def load_wp_chunk(ci):
                ra, rb = WCHUNKS[ci]
                for k, (ks, kn) in enumerate(((0, K0), (K0, K1))):
                    nc.sync.dma_start(
                        wp_sb[k][:, ra * PW:rb * PW],
                        wp_d[ks:ks + kn, ra * PW:rb * PW])

            def load_c1(band, eng):
                tiles = []
                for k, (ks, kn) in enumerate(((0, K0), (K0, K1))):
                    t = c1_pool.tile([kn, TPB * 128], bf16, tag=f"c1_{k}")
                    eng.dma_start(
                        t[:], c1_d[ks:ks + kn,
                                   band * TPB * 128:(band + 1) * TPB * 128])
                    tiles.append(t)
                return tiles

            # band n's halo rows [16n, 16n+24) live in chunks with
            # ra < 16n+24; pace loads two bands ahead of compute
            def wmax(n):
                return max(ci for ci, (ra, rb) in enumerate(WCHUNKS)
                           if ra < 16 * n + 24)

            # k0 halves first so band 0's k0-first matmul lead-in can
            # start as early as possible
            def load_wp_half(ci, k):
                ra, rb = WCHUNKS[ci]
                ks, kn = ((0, K0), (K0, K1))[k]
                nc.sync.dma_start(wp_sb[k][:, ra * PW:rb * PW],
                                  wp_d[ks:ks + kn, ra * PW:rb * PW])

            def load_c1_half(band, k, tiles):
                ks, kn = ((0, K0), (K0, K1))[k]
                t = c1_pool.tile([kn, TPB * 128], bf16, tag=f"c1_{k}")
                nc.scalar.dma_start(
                    t[:], c1_d[ks:ks + kn,
                               band * TPB * 128:(band + 1) * TPB * 128])
                tiles.append(t)

            b0 = []
            load_wp_half(0, 0)
            load_wp_half(1, 0)
            load_c1_half(0, 0, b0)
            load_wp_half(0, 1)
            load_wp_half(1, 1)
            load_c1_half(0, 1, b0)
            c1_tiles = {0: b0}
            c1_tiles[1] = load_c1(1, nc.scalar)
            load_wp_chunk(2)
            c1_tiles[2] = load_c1(2, nc.scalar)
            wp_next = 3

            # PE warm-up burst during the initial DMA window keeps the HAM
            # clock up before the real stream starts.
            warm = c1_pool.tile([128, 512], bf16, tag="warm")
            nc.gpsimd.memset(warm[:], 0.0)
            for _ in range(10):
                ps_w = ps_pool.tile([128, 1024], f32, tag="ps")
                nc.tensor.matmul(ps_w[:, 0:512], warm[:, 0:128],
                                 warm[:, 0:512], start=True, stop=True)

            # one persistent staged tile = ring of 4 per-band slots;
            # range-granular deps mean band b's copies wait only on band
            # b-4's gout reads, not a whole pool buffer (no group stall)
            SLOT = NPAIR * 2 * NCOL
            staged = st_pool.tile([128, 4 * SLOT], bf16, tag="staged")
            sap0 = staged[:]
            srow = sap0.ap[0][0]
            gap = go_d[:]

            def emit_wave(grp_, gbase_, plo, phi, sel):
                # one gout DMA per row-group a covering group-pair range
                # [plo, phi); runs of 2*WIN=288 elements (576B)
                npr = phi - plo
                for a in range(TH):
                    src = AP(sap0.tensor,
                             sap0.offset + gbase_ + plo * 2 * NCOL
                             + a * srow + 2 * a * HWW,
                             [[TH * srow, TW], [2 * NCOL, npr],
                              [1, 2 * WIN]])
                    dst = AP(gap.tensor,
                             gap.offset + (grp_ * TH + a) * GROW
                             + plo * 2 * WIN,
                             [[GPAIR * 2 * WIN, TW], [2 * WIN, npr],
                              [1, 2 * WIN]])
                    sel(a).dma_start(dst, src)

            for grp in range(NGRP):
                gbase = (grp % 2) * 2 * SLOT  # slots {0,1} or {2,3}

                for bb in range(GB):
                    band = grp * GB + bb
                    r0 = band * TH
                    c1_sb = c1_tiles.pop(band)
                    # prefetch three bands ahead; pace wp chunks likewise
                    if band + 3 < BANDS:
                        c1_tiles[band + 3] = load_c1(band + 3, nc.scalar)
                    while wp_next <= wmax(min(band + 2, BANDS - 1)):
                        load_wp_chunk(wp_next)
                        wp_next += 1

                    def mm(ps, tp, hf, k, start, stop):
                        t_i = 2 * tp + hf
                        c0 = t_i * TW
                        kn = (K0, K1)[k]
                        a1 = c1_sb[k][:]
                        lhsT = AP(a1.tensor, a1.offset + t_i * 128,
                                  [[a1.ap[0][0], kn], [1, 128]])
                        a2 = wp_sb[k][:]
                        rhs = AP(a2.tensor, a2.offset + r0 * PW + c0,
                                 [[a2.ap[0][0], kn], [PW, HH], [1, HWW]])
                        nc.tensor.matmul(
                            ps[:, hf * 512:hf * 512 + NCOL], lhsT, rhs,
                            start=start, stop=stop)

                    def docopy(ps, tp):
                        # one copy moves both tiles' grams n-major/
                        # hf-inner: staged col j = 2n + hf (interleaved)
                        pap = ps[:]
                        src2 = AP(pap.tensor, pap.offset,
                                  [[pap.ap[0][0], 128], [1, NCOL],
                                   [512, 2]])
                        d0 = gbase + (bb * NPAIR + tp) * 2 * NCOL
                        dst2 = staged[:, d0:d0 + 2 * NCOL]
                        if tp % 5 < 3:
                            nc.vector.tensor_copy(dst2, src2)
                        else:
                            nc.scalar.copy(dst2, src2)

                    if band == 0:
                        # k0-first lead-in: the k1 chunks land later on
                        # the rings, so run pairs 0-3's k0 matmuls first
                        # to start the PE earlier
                        NLEAD = 4
                        pss = []
                        for tp in range(NLEAD):
                            ps = ps_pool.tile([128, 1024], f32, tag="ps")
                            pss.append(ps)
                            for hf in range(2):
                                mm(ps, tp, hf, 0, True, False)
                        for tp in range(NLEAD):
                            ps = pss[tp]
                            for hf in range(2):
                                mm(ps, tp, hf, 1, False, True)
                            docopy(ps, tp)
                        tprange = range(NLEAD, NPAIR)
                    else:
                        tprange = range(NPAIR)

                    for tp in tprange:
                        ps = ps_pool.tile([128, 1024], f32, tag="ps")
                        for hf in range(2):
                            for k in range(2):
                                mm(ps, tp, hf, k, k == 0, k == 1)
                        docopy(ps, tp)
                        if band == BANDS - 1 and tp == 4:
                            # band-7 first half-wave mid-band: the drain
                            # tail shrinks by ~half a band
                            emit_wave(grp, gbase, NPAIR, NPAIR + 5,
                                      lambda a: (nc.sync,
                                                 nc.gpsimd)[a % 2])

                    if band < 4:
                        # Q7 is otherwise idle here: per-band waves free
                        # the staged slots ~10us earlier, removing the
                        # slot-WAR stalls that gate bands 4-6's copies
                        emit_wave(grp, gbase, bb * NPAIR,
                                  (bb + 1) * NPAIR,
                                  lambda a: nc.sync if a % 8 == 2
                                  else nc.gpsimd)
                    elif grp < NGRP - 1:
                        if bb == GB - 1:
                            # 2-band wave at group end (92KB per a-DMA
                            # amortizes the ~0.65us Q7 issue; sustained
                            # per-band waves saturate the Q7 sequencer).
                            # Bounded sync share only: the scalar
                            # sequencer is the ACT copy engine and must
                            # stay gout-free until the final band.
                            # reads are nearly drained by now: the sync
                            # ring takes half, relieving the backlogged
                            # SWDGE FIFO whose serial drain sets the tail
                            emit_wave(grp, gbase, 0, GPAIR,
                                      lambda a: nc.sync if a % 2 == 0
                                      else nc.gpsimd)
                    elif bb == 0:
                        # band 6: wave right after its copies, draining
                        # through band 7's compute
                        emit_wave(grp, gbase, 0, NPAIR,
                                  lambda a: (nc.sync, nc.gpsimd,
                                             nc.sync, nc.gpsimd,
                                             nc.gpsimd, nc.sync)[a % 6])
                    else:
                        # band 7 second half-wave: last copies done, both
                        # HWDGE rings free; even split keeps the two
                        # sequencers' serial issue chains balanced
                        emit_wave(grp, gbase, NPAIR + 5, GPAIR,
                                  lambda a: (nc.sync, nc.scalar)[a % 2])

                # gout waves: 16 a-group DMAs per 2-band group (92KB per
                # DMA amortizes the ~0.65us Q7 issue; 2-band granularity
                # keeps total Q7 issue load at ~32us -- per-band waves
                # saturate the Q7 sequencer and regress)
                gap = go_d[:]
                if grp == NGRP - 1:
                    # per-band: band 6 on sync+Q7; band 7 (no copies
                    # follow) may also use scalar. An HWDGE gout DMA
                    # blocks its ring's sequencer in program order, and
                    # the scalar sequencer is also the ACT copy engine.
                    for bb2 in range(GB):
                        if bb2 == 0:
                            rings = (nc.sync, nc.gpsimd, nc.sync,
                                     nc.gpsimd, nc.gpsimd, nc.sync)
                        else:
                            rings = (nc.sync, nc.scalar, nc.gpsimd)
                        for a in range(TH):
                            src = AP(sap0.tensor,
                                     sap0.offset + gbase + a * srow
                                     + 2 * a * HWW
                                     + bb2 * NPAIR * 2 * NCOL,
                                     [[TH * srow, TW], [2 * NCOL, NPAIR],
                                      [1, 2 * WIN]])
                            dst = AP(gap.tensor,
                                     gap.offset + (grp * TH + a) * GROW
                                     + bb2 * NPAIR * 2 * WIN,
                                     [[GPAIR * 2 * WIN, TW],
                                      [2 * WIN, NPAIR], [1, 2 * WIN]])
                            rings[a % len(rings)].dma_start(dst, src)
                else:
                    for a in range(TH):
                        src = AP(sap0.tensor,
                                 sap0.offset + gbase + a * srow
                                 + 2 * a * HWW,
                                 [[TH * srow, TW], [2 * NCOL, GPAIR],
                                  [1, 2 * WIN]])
                        dst = AP(gap.tensor,
                                 gap.offset + (grp * TH + a) * GROW,
                                 [[GPAIR * 2 * WIN, TW], [2 * WIN, GPAIR],
                                  [1, 2 * WIN]])
                        # sync-ring share only (its sequencer carries
                        # just wpad loads, prefetched 2-3 bands ahead so
                        # a bounded DMA-issue block is tolerable; the
                        # scalar sequencer is the ACT copy engine and
                        # must stay gout-free until the final band)
                        eng = nc.sync if a % 3 == 2 else nc.gpsimd
                        eng.dma_start(dst, src)

    nc.finalize()
    _CACHE["nc"] = nc
    return nc


def kernel(c1, warped, alpha):
    import sys
    if "/opt/trn_rl_repo" not in sys.path:
        sys.path.insert(0, "/opt/trn_rl_repo")
    import ml_dtypes
    from concourse.bass_utils import run_bass_kernel_spmd

    nc = _build()
    bf = ml_dtypes.bfloat16

    in_maps = []
    for b in range(B):
        wpad = np.zeros((C, PH, PW), np.float32)
        wpad[:, R:R + H, R:R + W] = warped[b]
        # tile c1: [C, band, a, t, b8] -> [C, band, t, b8, a]; m = b8*16 + a
        c1t = np.asarray(c1[b]).reshape(C, BANDS, TH, TPB, TW)
        c1t = np.ascontiguousarray(c1t.transpose(0, 1, 3, 4, 2))
        in_maps.append({
            "c1b": c1t.reshape(C, H * W).astype(bf),
            "wpad": wpad.reshape(C, PH * PW).astype(bf),
        })

    import os
    trace = bool(int(os.environ.get("COSTVOL_TRACE", "0")))
    res = run_bass_kernel_spmd(nc, in_maps, core_ids=list(range(B)),
                               trace=trace)
    if trace:
        _CACHE["last_exec_time_ns"] = res.exec_time_ns

    # host-side: de-interleave + diagonal gather + mean + PReLU
    a_val = float(np.asarray(alpha).reshape(-1)[0])
    dy, dx = np.meshgrid(np.arange(9), np.arange(9), indexing="ij")
    oidx = (dy * HWW + dx).reshape(-1)                      # [81]
    # gout row (grp*16+a) cols: [b8][pair(bb,tp)][j], j=2*(16dy+b8+dx)+hf
    jidx = (2 * (np.arange(TW)[:, None, None] + oidx[None, None, :])
            + np.arange(2)[None, :, None])                  # [b8, hf, 81]
    jflat = jidx.reshape(TW, 2 * 81)                        # [b8, 162]

    out = np.empty((B, 81, H, W), np.float32)
    for b in range(B):
        g = np.asarray(res.results[b]["gout"]).astype(np.float32)
        g = g.reshape(NGRP, TH, TW, GB, NPAIR, 2 * WIN)
        got = np.take_along_axis(
            g, jflat[None, None, :, None, None, :], axis=5)
        got = got.reshape(NGRP, TH, TW, GB, NPAIR, 2, 81)
        # axes [grp, a, b8, bb, tp, hf, o] -> [o, grp, bb, a, tp, hf, b8]
        cost = got.transpose(6, 0, 3, 1, 4, 5, 2).reshape(81, H, W) \
            * (1.0 / C)
        out[b] = np.where(cost >= 0, cost, a_val * cost)
    return out
